# revision 1
# baseline (speedup 1.0000x reference)
import numpy as np

# nn_AsyncCKCNN — self-contained kernel.
# Strategy: pure data parallelism over batch B=32 -> 8 NeuronCores x 4 samples
# (params replicated), executed via jax pmap on the neuron PJRT devices.
# A bit-faithful numpy implementation is kept as a fallback path.

W0 = 30.0
EPS = 1e-12
B, N, P, Cin, H, Cout, D = 32, 512, 257, 32, 64, 32, 16
NCORES = 8


def _leaky(x):
    return np.where(x >= 0, x, 0.01 * x)


def _groupnorm_np(h, g, b):
    mu = h.mean(axis=(1, 2), keepdims=True)
    var = h.var(axis=(1, 2), keepdims=True)
    return (h - mu) / np.sqrt(var + EPS) * g[None, :, None] + b[None, :, None]


def _kernel_numpy(inp):
    x = np.asarray(inp["x"], np.float32)
    positions = np.asarray(inp["positions"], np.float32)
    feat = x[..., 0].astype(np.int32)
    val = x[..., 1]
    t = x[..., 2]
    valid = np.any(x != 0.0, axis=-1)

    k1_w1 = inp["k1_w1"]; k1_b1 = inp["k1_b1"]
    k1_w2 = inp["k1_w2"]; k1_b2 = inp["k1_b2"]
    k1_w3 = inp["k1_w3"]; k1_b3 = inp["k1_b3"]
    bias1 = inp["bias1"]
    k2_w1 = inp["k2_w1"]; k2_b1 = inp["k2_b1"]
    k2_w2 = inp["k2_w2"]; k2_b2 = inp["k2_b2"]
    k2_w3 = inp["k2_w3"]; k2_b3 = inp["k2_b3"]
    bias2 = inp["bias2"]
    ln1_g = inp["ln1_g"]; ln1_b = inp["ln1_b"]
    ln2_g = inp["ln2_g"]; ln2_b = inp["ln2_b"]
    lin_w = inp["lin_w"]; lin_b = inp["lin_b"]

    W3r = k1_w3.reshape(D, H, Cin)            # (D,H,Cin)
    b3r = k1_b3.reshape(H, Cin)               # (H,Cin)
    W3m = W3r.transpose(1, 0, 2).reshape(H, D * Cin)  # (H, D*Cin)

    h1_all = np.empty((B, H, P), np.float32)
    for b in range(B):
        rel = (t[b][None, :] - positions[:, None]) / positions[-1]   # (P,N)
        mask = (rel <= 0.0) & valid[b][None, :]
        h = np.sin(W0 * (rel[..., None] * k1_w1[0] + k1_b1))         # (P,N,D)
        h = np.sin(W0 * (h @ k1_w2 + k1_b2))                         # (P,N,D)
        w = np.where(mask, val[b][None, :], 0.0).astype(np.float32)  # (P,N)
        hw = h * w[..., None]                                        # (P,N,D)
        O = (feat[b][:, None] == np.arange(Cin)[None, :]).astype(np.float32)  # (N,Cin)
        # G[p,d,c] = sum_n hw[p,n,d] O[n,c]
        G = np.einsum('pnd,nc->pdc', hw, O, optimize=True).reshape(P, D * Cin)
        h1 = G @ W3m.T                                               # (P,H)
        b3v = O @ b3r.T                                              # (N,H)
        h1 = h1 + w @ b3v                                            # (P,H)
        h1_all[b] = h1.T + bias1[:, None]
    h1_all = _leaky(_groupnorm_np(h1_all, ln1_g, ln1_b)).astype(np.float32)

    # CKConv kernel on the regular grid
    rel2 = np.linspace(0.0, 1.0, P, dtype=np.float32)[:, None]
    k = np.sin(W0 * (rel2 @ k2_w1 + k2_b1))
    k = np.sin(W0 * (k @ k2_w2 + k2_b2))
    k = (k @ k2_w3 + k2_b3).reshape(P, H, H)        # A[l, o, i]: lag-l coeff

    # causal conv h2[b,o,p] = sum_{l<=p} A[l,o,i] h1[b,i,p-l] via FFT (linear conv)
    nfft = 1024
    Hf = np.fft.rfft(h1_all.astype(np.float64), n=nfft, axis=2)     # (B,H,F)
    Af = np.fft.rfft(k.astype(np.float64), n=nfft, axis=0)          # (F,H,H)
    H2f = np.einsum('bif,foi->bof', Hf, Af, optimize=True)
    h2 = np.fft.irfft(H2f, n=nfft, axis=2)[..., :P].astype(np.float32)
    h2 = h2 + bias2[None, :, None]

    z = _leaky(_groupnorm_np(h2, ln2_g, ln2_b) + h1_all)
    y_last = lin_w @ z[:, :, -1].T.astype(np.float32)               # wrong orient fix below
    y_last = (z[:, :, -1] @ lin_w.T) + lin_b[None, :]               # (B,Cout)
    return y_last.astype(np.float32)


def _build_jax_fn():
    import jax
    import jax.numpy as jnp

    def per_shard(x, positions, k1_w1, k1_b1, k1_w2, k1_b2, k1_w3, k1_b3, bias1,
                  k2_w1, k2_b1, k2_w2, k2_b2, k2_w3, k2_b3, bias2,
                  ln1_g, ln1_b, ln2_g, ln2_b, lin_w, lin_b):
        feat = x[..., 0].astype(jnp.int32)
        val = x[..., 1]
        t = x[..., 2]
        valid = jnp.any(x != 0.0, axis=-1)

        rel = (t[:, None, :] - positions[None, :, None]) / positions[-1]
        mask = (rel <= 0.0) & valid[:, None, :]
        h = jnp.sin(W0 * (rel[..., None] * k1_w1[0] + k1_b1))
        h = jnp.sin(W0 * (h @ k1_w2 + k1_b2))
        w = jnp.where(mask, val[:, None, :], 0.0)
        hw = h * w[..., None]
        O = (feat[..., None] == jnp.arange(Cin)[None, None, :]).astype(jnp.float32)  # (b,N,Cin)
        W3r = k1_w3.reshape(D, H, Cin)
        b3r = k1_b3.reshape(H, Cin)
        G = jnp.einsum('bpnd,bnc->bpdc', hw, O)                     # (b,P,D,Cin)
        h1 = jnp.einsum('bpdc,dhc->bhp', G, W3r)
        b3v = jnp.einsum('bnc,hc->bnh', O, b3r)                     # (b,N,H)
        h1 = h1 + jnp.einsum('bpn,bnh->bhp', w, b3v) + bias1[None, :, None]

        def gn(hh, g, bb):
            mu = jnp.mean(hh, axis=(1, 2), keepdims=True)
            var = jnp.var(hh, axis=(1, 2), keepdims=True)
            return (hh - mu) * jax.lax.rsqrt(var + EPS) * g[None, :, None] + bb[None, :, None]

        h1 = jax.nn.leaky_relu(gn(h1, ln1_g, ln1_b))

        rel2 = jnp.linspace(0.0, 1.0, P)[:, None]
        k = jnp.sin(W0 * (rel2 * k2_w1[0] + k2_b1))
        k = jnp.sin(W0 * (k @ k2_w2 + k2_b2))
        k = k @ k2_w3 + k2_b3                                       # (P, H*H)
        ker = k.reshape(P, H, H).transpose(1, 2, 0)[..., ::-1]
        h2 = jax.lax.conv_general_dilated(h1, ker, (1,), [(P - 1, 0)],
                                          dimension_numbers=('NCH', 'OIH', 'NCH'))
        h2 = h2 + bias2[None, :, None]
        z = jax.nn.leaky_relu(gn(h2, ln2_g, ln2_b) + h1)
        y = jnp.einsum('oc,bc->bo', lin_w, z[:, :, -1]) + lin_b[None, :]
        return y

    return per_shard


_PMAPPED = None


def _kernel_jax_neuron(inp):
    import jax
    devs = jax.devices()
    if len(devs) < NCORES:
        raise RuntimeError(f"need {NCORES} devices, have {len(devs)}")
    global _PMAPPED
    if _PMAPPED is None:
        fn = _build_jax_fn()
        _PMAPPED = jax.pmap(
            fn,
            in_axes=(0,) + (None,) * 21,
            devices=devs[:NCORES],
        )
    x = np.asarray(inp["x"], np.float32).reshape(NCORES, B // NCORES, N, 3)
    args = [x] + [np.asarray(inp[k], np.float32) for k in (
        "positions",
        "k1_w1", "k1_b1", "k1_w2", "k1_b2", "k1_w3", "k1_b3", "bias1",
        "k2_w1", "k2_b1", "k2_w2", "k2_b2", "k2_w3", "k2_b3", "bias2",
        "ln1_g", "ln1_b", "ln2_g", "ln2_b", "lin_w", "lin_b")]
    y = _PMAPPED(*args)
    return np.asarray(y, np.float32).reshape(B, Cout)


def kernel(**inputs):
    try:
        return _kernel_jax_neuron(inputs)
    except Exception:
        return _kernel_numpy(inputs)


# revision 2
# speedup vs baseline: 1.0709x; 1.0709x over previous
import numpy as np

# nn_AsyncCKCNN — self-contained kernel.
# Strategy: pure data parallelism over batch B=32 -> 8 NeuronCores x 4 samples
# (params replicated), executed via jax pmap on the neuron PJRT devices.
# A bit-faithful numpy implementation is kept as a fallback path.

W0 = 30.0
EPS = 1e-12
B, N, P, Cin, H, Cout, D = 32, 512, 257, 32, 64, 32, 16
NCORES = 8


def _leaky(x):
    return np.where(x >= 0, x, 0.01 * x)


def _groupnorm_np(h, g, b):
    mu = h.mean(axis=(1, 2), keepdims=True)
    var = h.var(axis=(1, 2), keepdims=True)
    return (h - mu) / np.sqrt(var + EPS) * g[None, :, None] + b[None, :, None]


def _kernel_numpy(inp):
    x = np.asarray(inp["x"], np.float32)
    positions = np.asarray(inp["positions"], np.float32)
    feat = x[..., 0].astype(np.int32)
    val = x[..., 1]
    t = x[..., 2]
    valid = np.any(x != 0.0, axis=-1)

    k1_w1 = inp["k1_w1"]; k1_b1 = inp["k1_b1"]
    k1_w2 = inp["k1_w2"]; k1_b2 = inp["k1_b2"]
    k1_w3 = inp["k1_w3"]; k1_b3 = inp["k1_b3"]
    bias1 = inp["bias1"]
    k2_w1 = inp["k2_w1"]; k2_b1 = inp["k2_b1"]
    k2_w2 = inp["k2_w2"]; k2_b2 = inp["k2_b2"]
    k2_w3 = inp["k2_w3"]; k2_b3 = inp["k2_b3"]
    bias2 = inp["bias2"]
    ln1_g = inp["ln1_g"]; ln1_b = inp["ln1_b"]
    ln2_g = inp["ln2_g"]; ln2_b = inp["ln2_b"]
    lin_w = inp["lin_w"]; lin_b = inp["lin_b"]

    W3r = k1_w3.reshape(D, H, Cin)            # (D,H,Cin)
    b3r = k1_b3.reshape(H, Cin)               # (H,Cin)
    W3m = W3r.transpose(1, 0, 2).reshape(H, D * Cin)  # (H, D*Cin)

    h1_all = np.empty((B, H, P), np.float32)
    for b in range(B):
        rel = (t[b][None, :] - positions[:, None]) / positions[-1]   # (P,N)
        mask = (rel <= 0.0) & valid[b][None, :]
        h = np.sin(W0 * (rel[..., None] * k1_w1[0] + k1_b1))         # (P,N,D)
        h = np.sin(W0 * (h @ k1_w2 + k1_b2))                         # (P,N,D)
        w = np.where(mask, val[b][None, :], 0.0).astype(np.float32)  # (P,N)
        hw = h * w[..., None]                                        # (P,N,D)
        O = (feat[b][:, None] == np.arange(Cin)[None, :]).astype(np.float32)  # (N,Cin)
        # G[p,d,c] = sum_n hw[p,n,d] O[n,c]
        G = np.einsum('pnd,nc->pdc', hw, O, optimize=True).reshape(P, D * Cin)
        h1 = G @ W3m.T                                               # (P,H)
        b3v = O @ b3r.T                                              # (N,H)
        h1 = h1 + w @ b3v                                            # (P,H)
        h1_all[b] = h1.T + bias1[:, None]
    h1_all = _leaky(_groupnorm_np(h1_all, ln1_g, ln1_b)).astype(np.float32)

    # CKConv kernel on the regular grid
    rel2 = np.linspace(0.0, 1.0, P, dtype=np.float32)[:, None]
    k = np.sin(W0 * (rel2 @ k2_w1 + k2_b1))
    k = np.sin(W0 * (k @ k2_w2 + k2_b2))
    k = (k @ k2_w3 + k2_b3).reshape(P, H, H)        # A[l, o, i]: lag-l coeff

    # causal conv h2[b,o,p] = sum_{l<=p} A[l,o,i] h1[b,i,p-l] via FFT (linear conv)
    nfft = 1024
    Hf = np.fft.rfft(h1_all.astype(np.float64), n=nfft, axis=2)     # (B,H,F)
    Af = np.fft.rfft(k.astype(np.float64), n=nfft, axis=0)          # (F,H,H)
    H2f = np.einsum('bif,foi->bof', Hf, Af, optimize=True)
    h2 = np.fft.irfft(H2f, n=nfft, axis=2)[..., :P].astype(np.float32)
    h2 = h2 + bias2[None, :, None]

    z = _leaky(_groupnorm_np(h2, ln2_g, ln2_b) + h1_all)
    y_last = lin_w @ z[:, :, -1].T.astype(np.float32)               # wrong orient fix below
    y_last = (z[:, :, -1] @ lin_w.T) + lin_b[None, :]               # (B,Cout)
    return y_last.astype(np.float32)


def _build_jax_fn():
    import jax
    import jax.numpy as jnp

    def per_shard(x, positions, k1_w1, k1_b1, k1_w2, k1_b2, k1_w3, k1_b3, bias1,
                  k2_w1, k2_b1, k2_w2, k2_b2, k2_w3, k2_b3, bias2,
                  ln1_g, ln1_b, ln2_g, ln2_b, lin_w, lin_b):
        feat = x[..., 0].astype(jnp.int32)
        val = x[..., 1]
        t = x[..., 2]
        valid = jnp.any(x != 0.0, axis=-1)

        tmax = positions[-1]
        u = t / tmax                                    # (b,N)
        pn = positions / tmax                           # (P,)

        relT = u[:, :, None] - pn[None, None, :]        # (b,N,P)
        wT = jnp.where((relT <= 0.0) & valid[:, :, None], val[:, :, None], 0.0)

        # layer-1 SIREN factored: sin(a*u_n + c - a*pn_p) = sinα cosβ - cosα sinβ
        a1 = W0 * k1_w1[0]                              # (D,)
        alpha = u[:, :, None] * a1 + W0 * k1_b1         # (b,N,D)
        beta = pn[:, None] * a1                         # (P,D)
        U = jnp.concatenate([jnp.sin(alpha), jnp.cos(alpha)], axis=-1)   # (b,N,2D)
        V = W0 * jnp.concatenate(
            [jnp.cos(beta)[:, :, None] * k1_w2[None, :, :],
             -jnp.sin(beta)[:, :, None] * k1_w2[None, :, :]], axis=1)    # (P,2D,D)
        phi = jnp.einsum('bnk,pke->bnpe', U, V) + W0 * k1_b2             # (b,N,P,D)
        g = jnp.sin(phi)
        hw = g * wT[..., None]                          # (b,N,P,D)

        O = (feat[..., None] == jnp.arange(Cin)[None, None, :]).astype(jnp.float32)
        W3r = k1_w3.reshape(D, H, Cin)
        b3r = k1_b3.reshape(H, Cin)
        G = jnp.einsum('bnpe,bnc->bpec', hw, O)         # (b,P,D,Cin)
        h1 = jnp.einsum('bpec,ehc->bhp', G, W3r)
        b3v = jnp.einsum('bnc,hc->bnh', O, b3r)
        h1 = h1 + jnp.einsum('bnp,bnh->bhp', wT, b3v) + bias1[None, :, None]

        def gn(hh, gg, bb):
            mu = jnp.mean(hh, axis=(1, 2), keepdims=True)
            var = jnp.var(hh, axis=(1, 2), keepdims=True)
            return (hh - mu) * jax.lax.rsqrt(var + EPS) * gg[None, :, None] + bb[None, :, None]

        h1 = jax.nn.leaky_relu(gn(h1, ln1_g, ln1_b))

        # CKConv: ker[o,i,lag] = sum_d kk[lag,d] M_d[o,i] + b3m[o,i]  (rank D+1 in lag)
        rel2 = jnp.linspace(0.0, 1.0, P)[:, None]
        kk = jnp.sin(W0 * (rel2 * k2_w1[0] + k2_b1))
        kk = jnp.sin(W0 * (kk @ k2_w2 + k2_b2))         # (P,D)
        kaug = jnp.concatenate([kk, jnp.ones((P, 1))], axis=-1)          # (P,D+1)
        Maug = jnp.concatenate([k2_w3.reshape(D, H, H),
                                k2_b3.reshape(1, H, H)], axis=0)         # (D+1,H,H) [d,o,i]
        idx = jnp.arange(P)[None, :] - jnp.arange(P)[:, None]            # (q,p) = p-q
        Tmat = jnp.where(idx[..., None] >= 0,
                         kaug[jnp.clip(idx, 0), :], 0.0)                 # (q,p,D+1)
        C = jnp.einsum('biq,qpd->bidp', h1, Tmat)       # (b,H,D+1,P)
        h2 = jnp.einsum('bidp,doi->bop', C, Maug)       # (b,H,P)
        h2 = h2 + bias2[None, :, None]
        z = jax.nn.leaky_relu(gn(h2, ln2_g, ln2_b) + h1)
        y = jnp.einsum('oc,bc->bo', lin_w, z[:, :, -1]) + lin_b[None, :]
        return y

    return per_shard


_PMAPPED = None


def _kernel_jax_neuron(inp):
    import jax
    devs = jax.devices()
    if len(devs) < NCORES:
        raise RuntimeError(f"need {NCORES} devices, have {len(devs)}")
    global _PMAPPED
    if _PMAPPED is None:
        fn = _build_jax_fn()
        _PMAPPED = jax.pmap(
            fn,
            in_axes=(0,) + (None,) * 21,
            devices=devs[:NCORES],
        )
    x = np.asarray(inp["x"], np.float32).reshape(NCORES, B // NCORES, N, 3)
    args = [x] + [np.asarray(inp[k], np.float32) for k in (
        "positions",
        "k1_w1", "k1_b1", "k1_w2", "k1_b2", "k1_w3", "k1_b3", "bias1",
        "k2_w1", "k2_b1", "k2_w2", "k2_b2", "k2_w3", "k2_b3", "bias2",
        "ln1_g", "ln1_b", "ln2_g", "ln2_b", "lin_w", "lin_b")]
    y = _PMAPPED(*args)
    return np.asarray(y, np.float32).reshape(B, Cout)


def kernel(**inputs):
    try:
        return _kernel_jax_neuron(inputs)
    except Exception:
        return _kernel_numpy(inputs)


# revision 4
# speedup vs baseline: 1.1075x; 1.0342x over previous
import numpy as np

# nn_AsyncCKCNN — self-contained kernel.
# Strategy: pure data parallelism over batch B=32 -> 8 NeuronCores x 4 samples
# (params replicated), executed via jax pmap on the neuron PJRT devices.
# A bit-faithful numpy implementation is kept as a fallback path.

W0 = 30.0
EPS = 1e-12
B, N, P, Cin, H, Cout, D = 32, 512, 257, 32, 64, 32, 16
NCORES = 8


def _leaky(x):
    return np.where(x >= 0, x, 0.01 * x)


def _groupnorm_np(h, g, b):
    mu = h.mean(axis=(1, 2), keepdims=True)
    var = h.var(axis=(1, 2), keepdims=True)
    return (h - mu) / np.sqrt(var + EPS) * g[None, :, None] + b[None, :, None]


def _kernel_numpy(inp):
    x = np.asarray(inp["x"], np.float32)
    positions = np.asarray(inp["positions"], np.float32)
    feat = x[..., 0].astype(np.int32)
    val = x[..., 1]
    t = x[..., 2]
    valid = np.any(x != 0.0, axis=-1)

    k1_w1 = inp["k1_w1"]; k1_b1 = inp["k1_b1"]
    k1_w2 = inp["k1_w2"]; k1_b2 = inp["k1_b2"]
    k1_w3 = inp["k1_w3"]; k1_b3 = inp["k1_b3"]
    bias1 = inp["bias1"]
    k2_w1 = inp["k2_w1"]; k2_b1 = inp["k2_b1"]
    k2_w2 = inp["k2_w2"]; k2_b2 = inp["k2_b2"]
    k2_w3 = inp["k2_w3"]; k2_b3 = inp["k2_b3"]
    bias2 = inp["bias2"]
    ln1_g = inp["ln1_g"]; ln1_b = inp["ln1_b"]
    ln2_g = inp["ln2_g"]; ln2_b = inp["ln2_b"]
    lin_w = inp["lin_w"]; lin_b = inp["lin_b"]

    W3r = k1_w3.reshape(D, H, Cin)            # (D,H,Cin)
    b3r = k1_b3.reshape(H, Cin)               # (H,Cin)
    W3m = W3r.transpose(1, 0, 2).reshape(H, D * Cin)  # (H, D*Cin)

    h1_all = np.empty((B, H, P), np.float32)
    for b in range(B):
        rel = (t[b][None, :] - positions[:, None]) / positions[-1]   # (P,N)
        mask = (rel <= 0.0) & valid[b][None, :]
        h = np.sin(W0 * (rel[..., None] * k1_w1[0] + k1_b1))         # (P,N,D)
        h = np.sin(W0 * (h @ k1_w2 + k1_b2))                         # (P,N,D)
        w = np.where(mask, val[b][None, :], 0.0).astype(np.float32)  # (P,N)
        hw = h * w[..., None]                                        # (P,N,D)
        O = (feat[b][:, None] == np.arange(Cin)[None, :]).astype(np.float32)  # (N,Cin)
        # G[p,d,c] = sum_n hw[p,n,d] O[n,c]
        G = np.einsum('pnd,nc->pdc', hw, O, optimize=True).reshape(P, D * Cin)
        h1 = G @ W3m.T                                               # (P,H)
        b3v = O @ b3r.T                                              # (N,H)
        h1 = h1 + w @ b3v                                            # (P,H)
        h1_all[b] = h1.T + bias1[:, None]
    h1_all = _leaky(_groupnorm_np(h1_all, ln1_g, ln1_b)).astype(np.float32)

    # CKConv kernel on the regular grid
    rel2 = np.linspace(0.0, 1.0, P, dtype=np.float32)[:, None]
    k = np.sin(W0 * (rel2 @ k2_w1 + k2_b1))
    k = np.sin(W0 * (k @ k2_w2 + k2_b2))
    k = (k @ k2_w3 + k2_b3).reshape(P, H, H)        # A[l, o, i]: lag-l coeff

    # causal conv h2[b,o,p] = sum_{l<=p} A[l,o,i] h1[b,i,p-l] via FFT (linear conv)
    nfft = 1024
    Hf = np.fft.rfft(h1_all.astype(np.float64), n=nfft, axis=2)     # (B,H,F)
    Af = np.fft.rfft(k.astype(np.float64), n=nfft, axis=0)          # (F,H,H)
    H2f = np.einsum('bif,foi->bof', Hf, Af, optimize=True)
    h2 = np.fft.irfft(H2f, n=nfft, axis=2)[..., :P].astype(np.float32)
    h2 = h2 + bias2[None, :, None]

    z = _leaky(_groupnorm_np(h2, ln2_g, ln2_b) + h1_all)
    y_last = lin_w @ z[:, :, -1].T.astype(np.float32)               # wrong orient fix below
    y_last = (z[:, :, -1] @ lin_w.T) + lin_b[None, :]               # (B,Cout)
    return y_last.astype(np.float32)


def _build_jax_fn():
    import jax
    import jax.numpy as jnp

    def per_shard(x, positions, k1_w1, k1_b1, k1_w2, k1_b2, k1_w3, k1_b3, bias1,
                  k2_w1, k2_b1, k2_w2, k2_b2, k2_w3, k2_b3, bias2,
                  ln1_g, ln1_b, ln2_g, ln2_b, lin_w, lin_b):
        feat = x[..., 0].astype(jnp.int32)
        val = x[..., 1]
        t = x[..., 2]
        valid = jnp.any(x != 0.0, axis=-1)

        tmax = positions[-1]
        u = t / tmax                                    # (b,N)
        pn = positions / tmax                           # (P,)

        relT = u[:, :, None] - pn[None, None, :]        # (b,N,P)
        wT = jnp.where((relT <= 0.0) & valid[:, :, None], val[:, :, None], 0.0)

        # layer-1 SIREN factored: sin(a*u_n + c - a*pn_p) = sinα cosβ - cosα sinβ
        a1 = W0 * k1_w1[0]                              # (D,)
        alpha = u[:, :, None] * a1 + W0 * k1_b1         # (b,N,D)
        beta = pn[:, None] * a1                         # (P,D)
        U = jnp.concatenate([jnp.sin(alpha), jnp.cos(alpha)], axis=-1)   # (b,N,2D)
        V = W0 * jnp.concatenate(
            [jnp.cos(beta)[:, :, None] * k1_w2[None, :, :],
             -jnp.sin(beta)[:, :, None] * k1_w2[None, :, :]], axis=1)    # (P,2D,D)
        phi = jnp.einsum('bnk,pke->bnpe', U, V) + W0 * k1_b2             # (b,N,P,D)
        g = jnp.sin(phi)
        hw = g * wT[..., None]                          # (b,N,P,D)

        O = (feat[..., None] == jnp.arange(Cin)[None, None, :]).astype(jnp.bfloat16)
        W3r = k1_w3.reshape(D, H, Cin)
        b3r = k1_b3.reshape(H, Cin)
        G = jnp.einsum('bnpe,bnc->bpec', hw.astype(jnp.bfloat16), O,
                       preferred_element_type=jnp.float32)          # (b,P,D,Cin)
        O = O.astype(jnp.float32)
        h1 = jnp.einsum('bpec,ehc->bhp', G, W3r)
        b3v = jnp.einsum('bnc,hc->bnh', O, b3r)
        h1 = h1 + jnp.einsum('bnp,bnh->bhp', wT, b3v) + bias1[None, :, None]

        def gn(hh, gg, bb):
            mu = jnp.mean(hh, axis=(1, 2), keepdims=True)
            var = jnp.var(hh, axis=(1, 2), keepdims=True)
            return (hh - mu) * jax.lax.rsqrt(var + EPS) * gg[None, :, None] + bb[None, :, None]

        h1 = jax.nn.leaky_relu(gn(h1, ln1_g, ln1_b))

        # CKConv: ker[o,i,lag] = sum_d kk[lag,d] M_d[o,i] + b3m[o,i]  (rank D+1 in lag)
        rel2 = jnp.linspace(0.0, 1.0, P)[:, None]
        kk = jnp.sin(W0 * (rel2 * k2_w1[0] + k2_b1))
        kk = jnp.sin(W0 * (kk @ k2_w2 + k2_b2))         # (P,D)
        kaug = jnp.concatenate([kk, jnp.ones((P, 1))], axis=-1)          # (P,D+1)
        Maug = jnp.concatenate([k2_w3.reshape(D, H, H),
                                k2_b3.reshape(1, H, H)], axis=0)         # (D+1,H,H) [d,o,i]
        idx = jnp.arange(P)[None, :] - jnp.arange(P)[:, None]            # (q,p) = p-q
        Tmat = jnp.where(idx[..., None] >= 0,
                         kaug[jnp.clip(idx, 0), :], 0.0)                 # (q,p,D+1)
        C = jnp.einsum('biq,qpd->bidp', h1.astype(jnp.bfloat16),
                       Tmat.astype(jnp.bfloat16),
                       preferred_element_type=jnp.float32)          # (b,H,D+1,P)
        h2 = jnp.einsum('bidp,doi->bop', C.astype(jnp.bfloat16),
                        Maug.astype(jnp.bfloat16),
                        preferred_element_type=jnp.float32)         # (b,H,P)
        h2 = h2 + bias2[None, :, None]
        z = jax.nn.leaky_relu(gn(h2, ln2_g, ln2_b) + h1)
        y = jnp.einsum('oc,bc->bo', lin_w, z[:, :, -1]) + lin_b[None, :]
        return y

    return per_shard


_PMAPPED = None


def _kernel_jax_neuron(inp):
    import jax
    devs = jax.devices()
    if len(devs) < NCORES:
        raise RuntimeError(f"need {NCORES} devices, have {len(devs)}")
    global _PMAPPED
    if _PMAPPED is None:
        fn = _build_jax_fn()
        _PMAPPED = jax.pmap(
            fn,
            in_axes=(0,) + (None,) * 21,
            devices=devs[:NCORES],
        )
    x = np.asarray(inp["x"], np.float32).reshape(NCORES, B // NCORES, N, 3)
    args = [x] + [np.asarray(inp[k], np.float32) for k in (
        "positions",
        "k1_w1", "k1_b1", "k1_w2", "k1_b2", "k1_w3", "k1_b3", "bias1",
        "k2_w1", "k2_b1", "k2_w2", "k2_b2", "k2_w3", "k2_b3", "bias2",
        "ln1_g", "ln1_b", "ln2_g", "ln2_b", "lin_w", "lin_b")]
    y = _PMAPPED(*args)
    return np.asarray(y, np.float32).reshape(B, Cout)


def kernel(**inputs):
    try:
        return _kernel_jax_neuron(inputs)
    except Exception:
        return _kernel_numpy(inputs)
